# revision 12
# baseline (speedup 1.0000x reference)
"""3-layer GCN (GCNConv normalize=False, bias=False, aggr='add') on 8 TRN2 NeuronCores.

Math: each layer computes segment_sum((x @ W)[src], dst) which equals
segment_sum(x[src], dst) @ W (linear ops commute), so we aggregate the raw
features first (gather + scatter-add over edges) and then apply the tiny 64x64
weight to the 12500-row destination shard only.

Distribution (8 cores):
  - nodes sharded by id: core c owns dst rows [c*12500, (c+1)*12500)
  - edges assigned to the core owning their dst; within a core, edges are
    grouped by src shard (8 groups) so gather indices fit in int16 against a
    12500-row subtable, and sorted by dst inside each group
  - per layer: dma_gather rows of the full node-feature table (x for layer 0,
    AllGather output for layers 1/2) -> SBUF, dma_scatter_add into an agg
    table in DRAM, small GEMM agg @ W_l, AllGather the shard for the next
    layer. Final-layer shards are concatenated on the host.

SWDGE details: one gather/scatter instruction is limited to ~<=2032 indices
(descriptor ring is 128 slots/engine; descs = num_idxs/16+1), so edges are
chunked at 2016/instruction. Gathers run on SWDGE queues 0/1, scatters on
queues 2/3 into two separate agg copies (queue q -> Q7 core pair q; separate
agg copies keep concurrent scatter-add streams race-free). The GEMM sums the
two copies.
"""

import numpy as np

from concourse import bacc, bass, mybir, tile
from concourse import tile_sem_assignment as _tsa
from concourse.bass_utils import run_bass_kernel_spmd
from concourse.masks import make_identity


def _assign_tick_queue_aware(self, inst):
    """Queue-aware replacement for TileClockTick._assign_tick: SWDGE (Pool)
    DMA instructions get DMASW semaphore lanes partitioned by their SWDGE
    queue_num (lanes {2q, 2q+1} for queue q) instead of global round-robin.
    The runtime locks each DMASW semaphore to the first SWDGE queue that
    uses it, so cross-queue round-robin deadlocks/corrupts. Everything else
    is identical to the original (concourse/tile_sem_assignment.py)."""
    engine = inst.engine
    eng_proc_idx = (
        _tsa.ENGINE_SEQUENCER_TO_IDX if inst.is_sequencer_only()
        else _tsa.ENGINE_TO_IDX
    )[engine]
    if isinstance(inst, _tsa.DMAInst) and not isinstance(
        inst, _tsa.bass_isa.UserSyncedRemoteDMADescs
    ):
        if engine == _tsa.mybir.EngineType.Pool:
            if self.swdge_sem_count == _tsa.NUM_SWDGE_GLOBAL_SEMS:
                qn = int(getattr(inst, "queue_num", 0) or 0) % 4
                lanes_per_q = _tsa.NUM_SWDGE_GLOBAL_SEMS // 4
                tog = getattr(self, "_q_lane_toggle", None)
                if tog is None:
                    tog = self._q_lane_toggle = {}
                t = tog.get(qn, 0)
                tog[qn] = (t + 1) % lanes_per_q
                lane = qn * lanes_per_q + t
            else:
                lane = self.next_sw_dma_idx
                self.next_sw_dma_idx = (
                    self.next_sw_dma_idx + 1
                ) % self.swdge_sem_count
            inst_proc_idx = _tsa.PROC_NAME_TO_IDX[f"DMASW{lane}"]
        else:
            inst_proc_idx = _tsa.PROC_NAME_TO_IDX[f"DMAHW{self.next_hw_dma_idx}"]
            self.next_hw_dma_idx = (self.next_hw_dma_idx + 1) % _tsa.NUM_HWDGE_SEMS
    elif isinstance(inst, _tsa.mybir.InstCollectiveCompute):
        inst_proc_idx = _tsa.PROC_NAME_TO_IDX["Collectives"]
    else:
        inst_proc_idx = eng_proc_idx

    if not inst.is_executable():
        if not isinstance(inst, _tsa.BassTileCriticalSection):
            return
    if isinstance(inst, _tsa.bass_isa.InstPseudoReloadLibraryIndex):
        return

    if inst.descendants or isinstance(inst, _tsa._DMA_OR_COLLECTIVE_TYPES):
        inst.bass_scheduled_tick = self.global_clock.advance(inst_proc_idx)
        inst.bass_scheduled_proc = inst_proc_idx
        inst.bass_scheduled_scope = self.scope_name
        self._proc_insts[self.root_scope_name][inst_proc_idx].append(inst)
        if getattr(inst, "gen_mode", 0) == 1 and inst_proc_idx != eng_proc_idx:
            eng_tick = self.global_clock.advance(eng_proc_idx)
            self.tc.prep_eng_ticks[inst.name] = (eng_proc_idx, eng_tick)
            self._prep_eng_names[self.root_scope_name].append(inst.name)


_tsa.TileClockTick._assign_tick = _assign_tick_queue_aware

F32 = mybir.dt.float32
I16 = mybir.dt.int16

# Problem constants (hardcoded per harness contract).
N = 100000   # nodes
D = 64       # feature dim
E = 1200000  # edges
C = 8        # cores
NS = N // C  # node shard = 12500
G = 8        # src groups per core (= src shards)
E_CHUNK = 1792        # edges per SWDGE instruction
K_CHUNKS = 12         # chunks per (core, group): >= max dst multiplicity (11)
E_PAD = E_CHUNK * K_CHUNKS
AGG_ROWS = 12544      # 98 * 128 destination rows incl. padding/trash region
TRASH = NS            # scatter index used for padded edges (row 12500)
LAYERS = 3
N_AGG = 2             # agg copies; queues 0/1 gathers, 2/3 scatters


def build_graph(n=N, d=D, c=C, g=G, e_chunk=E_CHUNK, k_chunks=K_CHUNKS,
                layers=LAYERS):
    ns = n // c
    agg_rows = ((ns + 128) + 127) // 128 * 128
    n_tiles = agg_rows // 128
    e_pad = e_chunk * k_chunks
    ep16 = e_pad // 16
    ec16 = e_chunk // 16
    ecb = (e_chunk + 127) // 128  # msg blocks per chunk

    nc = bacc.Bacc("TRN2", target_bir_lowering=False, debug=False,
                   num_devices=c, num_swdge_queues=4)

    x_ext = nc.declare_dram_parameter("x", [n, d], F32, isOutput=False)
    g_ext = nc.declare_dram_parameter("gidx", [g, 128, ep16], I16, isOutput=False)
    s_ext = nc.declare_dram_parameter("sidx", [g, 128, ep16], I16, isOutput=False)
    w_ext = nc.declare_dram_parameter("w", [layers, d, d], F32, isOutput=False)
    out_ext = nc.declare_dram_parameter("out", [ns, d], F32, isOutput=True)

    aggs = [nc.dram_tensor(f"agg{q}", [agg_rows, d], F32) for q in range(N_AGG)]
    shards = [nc.dram_tensor(f"shard{l}", [ns, d], F32) for l in range(layers - 1)]
    tables = [
        nc.dram_tensor(f"table{l}", [n, d], F32, addr_space="Shared")
        for l in range(1, layers)
    ]

    with tile.TileContext(nc) as tc:
        with (
            tc.tile_pool(name="const", bufs=1) as cpool,
            tc.tile_pool(name="idx", bufs=1) as ipool,
            tc.tile_pool(name="msg", bufs=8) as mpool,
            tc.tile_pool(name="gemm", bufs=4) as gpool,
            tc.tile_pool(name="psum", bufs=4, space="PSUM") as ppool,
        ):
            ident = cpool.tile([128, 128], F32)
            make_identity(nc, ident[:])

            w_sb = cpool.tile([d, layers * d], F32)
            for l in range(layers):
                nc.sync.dma_start(out=w_sb[:, l * d:(l + 1) * d], in_=w_ext[l])

            zero_sb = cpool.tile([128, agg_rows * d // 128], F32)
            nc.vector.memset(zero_sb[:], 0.0)

            gidx_sb, sidx_sb = [], []
            for gi in range(g):
                gt = ipool.tile([128, ep16], I16, name=f"gidx{gi}")
                nc.sync.dma_start(out=gt[:], in_=g_ext[gi])
                st = ipool.tile([128, ep16], I16, name=f"sidx{gi}")
                nc.sync.dma_start(out=st[:], in_=s_ext[gi])
                gidx_sb.append(gt)
                sidx_sb.append(st)

            for l in range(layers):
                table = x_ext if l == 0 else tables[l - 1]

                for q in range(N_AGG):
                    nc.sync.dma_start(
                        out=aggs[q][:].rearrange("(p b) e -> p (b e)", p=128),
                        in_=zero_sb[:],
                    )

                chunks = [(gi, ki) for gi in range(g) for ki in range(k_chunks)]
                for r0 in range(0, len(chunks), 2):
                    round_chunks = chunks[r0:r0 + 2]
                    msgs = []
                    for q, (gi, ki) in enumerate(round_chunks):
                        msg = mpool.tile([128, ecb * d], F32,
                                         name="msg", tag=f"msg{q}")
                        nc.gpsimd.dma_gather(
                            out_ap=msg[:].rearrange("p (b e) -> p b e", e=d),
                            in_ap=table[gi * ns:(gi + 1) * ns, :],
                            idxs_ap=gidx_sb[gi][:, ki * ec16:(ki + 1) * ec16],
                            num_idxs=e_chunk,
                            num_idxs_reg=e_chunk,
                            elem_size=d,
                            queue_num=q,
                            single_packet=False,
                        )
                        msgs.append(msg)
                    for q, (gi, ki) in enumerate(round_chunks):
                        nc.gpsimd.dma_scatter_add(
                            out_ap=aggs[q][:, :],
                            in_ap=msgs[q][:].rearrange("p (b e) -> p b e", e=d),
                            idxs_ap=sidx_sb[gi][:, ki * ec16:(ki + 1) * ec16],
                            num_idxs=e_chunk,
                            num_idxs_reg=e_chunk,
                            elem_size=d,
                            queue_num=2 + q,
                            single_packet=False,
                        )

                dest = out_ext if l == layers - 1 else shards[l]
                for t in range(n_tiles):
                    a = gpool.tile([128, d], F32, name="a", tag="a")
                    nc.sync.dma_start(out=a[:], in_=aggs[0][t * 128:(t + 1) * 128, :])
                    for qq in range(1, N_AGG):
                        a1 = gpool.tile([128, d], F32, name="a1", tag="a1")
                        nc.sync.dma_start(out=a1[:], in_=aggs[qq][t * 128:(t + 1) * 128, :])
                        nc.any.tensor_add(out=a[:], in0=a[:], in1=a1[:])
                    at_ps = ppool.tile([d, 128], F32, name="atp", tag="atp")
                    nc.tensor.transpose(out=at_ps[:], in_=a[:], identity=ident[:])
                    at = gpool.tile([d, 128], F32, name="ats", tag="ats")
                    nc.vector.tensor_copy(out=at[:], in_=at_ps[:])
                    o_ps = ppool.tile([128, d], F32, name="ops", tag="ops")
                    nc.tensor.matmul(
                        out=o_ps[:],
                        lhsT=at[:],
                        rhs=w_sb[:, l * d:(l + 1) * d],
                        start=True,
                        stop=True,
                    )
                    o = gpool.tile([128, d], F32, name="o", tag="o")
                    nc.vector.tensor_copy(out=o[:], in_=o_ps[:])
                    rows = min(128, ns - t * 128)
                    if rows > 0:
                        nc.sync.dma_start(
                            out=dest[t * 128:t * 128 + rows, :], in_=o[:rows, :]
                        )

                if l < layers - 1:
                    nc.gpsimd.collective_compute(
                        "AllGather",
                        mybir.AluOpType.bypass,
                        replica_groups=[list(range(c))],
                        ins=[shards[l][:]],
                        outs=[tables[l][:]],
                    )

    return nc


def _wrap16(arr_1d):
    """Wrap a [e_pad] index vector to the [16, e_pad//16] layout used by the
    gather/scatter DMA instructions (index j at partition j%16, col j//16),
    replicated across the 8 Q7-core partition groups -> [128, e_pad//16].
    Chunk c of size 2016 occupies columns [c*126, (c+1)*126)."""
    w = arr_1d.reshape(-1, 16).T
    return np.tile(w, (8, 1))


def prep_inputs(x, edge_index, W0, W1, W2, n=N, d=D, c=C, g=G,
                e_chunk=E_CHUNK, k_chunks=K_CHUNKS):
    ns = n // c
    e_pad = e_chunk * k_chunks
    src = np.asarray(edge_index[0], dtype=np.int64)
    dst = np.asarray(edge_index[1], dtype=np.int64)
    core = dst // ns
    grp = src // ns
    cg = core * g + grp
    # Sort by (group, dst); within each dst run, occurrence k goes to chunk
    # (dst + k) % k_chunks so every chunk sees each dst at most once (the
    # SDMA CCE read-modify-write races on duplicate rows within one
    # instruction; chunks are serialized against each other).
    order0 = np.argsort(cg * np.int64(n) + dst, kind="stable")
    d0 = dst[order0]
    cg0 = cg[order0]
    run_key = cg0 * np.int64(n) + d0
    new_run = np.empty(len(d0), bool)
    new_run[0] = True
    np.not_equal(run_key[1:], run_key[:-1], out=new_run[1:])
    run_id = np.cumsum(new_run) - 1
    run_start = np.flatnonzero(new_run)
    rank = np.arange(len(d0)) - run_start[run_id]
    assert rank.max() < k_chunks, f"dst multiplicity {rank.max()+1} > k_chunks"
    chunk = (d0 + rank) % k_chunks
    order1 = np.argsort(cg0 * np.int64(k_chunks) + chunk, kind="stable")
    s_src = src[order0][order1]
    s_dst = d0[order1]
    s_cgk = (cg0 * k_chunks + chunk)[order1]
    counts = np.bincount(s_cgk, minlength=c * g * k_chunks)
    assert counts.max() <= e_chunk, f"chunk count {counts.max()} > {e_chunk}"
    offsets = np.zeros(c * g * k_chunks + 1, np.int64)
    np.cumsum(counts, out=offsets[1:])

    x = np.ascontiguousarray(np.asarray(x, dtype=np.float32))
    w = np.ascontiguousarray(np.stack([W0, W1, W2]).astype(np.float32))

    in_maps = []
    ep16 = e_pad // 16
    for ci in range(c):
        gidx = np.zeros((g, 128, ep16), np.int16)
        sidx = np.zeros((g, 128, ep16), np.int16)
        for gi in range(g):
            ga = np.zeros(e_pad, np.int16)
            sa = np.full(e_pad, ns, np.int16)  # trash row
            for ki in range(k_chunks):
                j = (ci * g + gi) * k_chunks + ki
                lo, hi = offsets[j], offsets[j + 1]
                nseg = hi - lo
                base = ki * e_chunk
                ga[base:base + nseg] = s_src[lo:hi] - gi * ns
                sa[base:base + nseg] = s_dst[lo:hi] - ci * ns
            gidx[gi] = _wrap16(ga)
            sidx[gi] = _wrap16(sa)
        in_maps.append({"x": x, "gidx": gidx, "sidx": sidx, "w": w})
    return in_maps


def kernel(x, edge_index, W0, W1, W2, _trace=False, _trace_kwargs=None):
    nc = build_graph()
    nc.compile()
    in_maps = prep_inputs(x, edge_index, W0, W1, W2)
    res = run_bass_kernel_spmd(
        nc, in_maps, core_ids=list(range(C)), trace=_trace,
        **(_trace_kwargs or {}),
    )
    out = np.concatenate([res.results[ci]["out"] for ci in range(C)], axis=0)
    if _trace:
        return out, res
    return out


# revision 13
# speedup vs baseline: 1.1259x; 1.1259x over previous
"""3-layer GCN (GCNConv normalize=False, bias=False, aggr='add') on 8 TRN2 NeuronCores.

Math: each layer computes segment_sum((x @ W)[src], dst) which equals
segment_sum(x[src], dst) @ W (linear ops commute), so we aggregate the raw
features first (gather + scatter-add over edges) and then apply the tiny 64x64
weight to the 12500-row destination shard only.

Distribution (8 cores):
  - nodes sharded by id: core c owns dst rows [c*12500, (c+1)*12500)
  - edges assigned to the core owning their dst; within a core, edges are
    grouped by src shard (8 groups) so gather indices fit in int16 against a
    12500-row subtable, and sorted by dst inside each group
  - per layer: dma_gather rows of the full node-feature table (x for layer 0,
    AllGather output for layers 1/2) -> SBUF, dma_scatter_add into an agg
    table in DRAM, small GEMM agg @ W_l, AllGather the shard for the next
    layer. Final-layer shards are concatenated on the host.

SWDGE details: one gather/scatter instruction is limited to ~<=2032 indices
(descriptor ring is 128 slots/engine; descs = num_idxs/16+1), so edges are
chunked at 2016/instruction. Gathers run on SWDGE queues 0/1, scatters on
queues 2/3 into two separate agg copies (queue q -> Q7 core pair q; separate
agg copies keep concurrent scatter-add streams race-free). The GEMM sums the
two copies.
"""

import numpy as np

from concourse import bacc, bass, mybir, tile
from concourse import tile_sem_assignment as _tsa
from concourse.bass_utils import run_bass_kernel_spmd
from concourse.masks import make_identity


def _assign_tick_queue_aware(self, inst):
    """Queue-aware replacement for TileClockTick._assign_tick: SWDGE (Pool)
    DMA instructions get DMASW semaphore lanes partitioned by their SWDGE
    queue_num (lanes {2q, 2q+1} for queue q) instead of global round-robin.
    The runtime locks each DMASW semaphore to the first SWDGE queue that
    uses it, so cross-queue round-robin deadlocks/corrupts. Everything else
    is identical to the original (concourse/tile_sem_assignment.py)."""
    engine = inst.engine
    eng_proc_idx = (
        _tsa.ENGINE_SEQUENCER_TO_IDX if inst.is_sequencer_only()
        else _tsa.ENGINE_TO_IDX
    )[engine]
    if isinstance(inst, _tsa.DMAInst) and not isinstance(
        inst, _tsa.bass_isa.UserSyncedRemoteDMADescs
    ):
        if engine == _tsa.mybir.EngineType.Pool:
            if self.swdge_sem_count == _tsa.NUM_SWDGE_GLOBAL_SEMS:
                qn = int(getattr(inst, "queue_num", 0) or 0) % 4
                lanes_per_q = _tsa.NUM_SWDGE_GLOBAL_SEMS // 4
                tog = getattr(self, "_q_lane_toggle", None)
                if tog is None:
                    tog = self._q_lane_toggle = {}
                t = tog.get(qn, 0)
                tog[qn] = (t + 1) % lanes_per_q
                lane = qn * lanes_per_q + t
            else:
                lane = self.next_sw_dma_idx
                self.next_sw_dma_idx = (
                    self.next_sw_dma_idx + 1
                ) % self.swdge_sem_count
            inst_proc_idx = _tsa.PROC_NAME_TO_IDX[f"DMASW{lane}"]
        else:
            inst_proc_idx = _tsa.PROC_NAME_TO_IDX[f"DMAHW{self.next_hw_dma_idx}"]
            self.next_hw_dma_idx = (self.next_hw_dma_idx + 1) % _tsa.NUM_HWDGE_SEMS
    elif isinstance(inst, _tsa.mybir.InstCollectiveCompute):
        inst_proc_idx = _tsa.PROC_NAME_TO_IDX["Collectives"]
    else:
        inst_proc_idx = eng_proc_idx

    if not inst.is_executable():
        if not isinstance(inst, _tsa.BassTileCriticalSection):
            return
    if isinstance(inst, _tsa.bass_isa.InstPseudoReloadLibraryIndex):
        return

    if inst.descendants or isinstance(inst, _tsa._DMA_OR_COLLECTIVE_TYPES):
        inst.bass_scheduled_tick = self.global_clock.advance(inst_proc_idx)
        inst.bass_scheduled_proc = inst_proc_idx
        inst.bass_scheduled_scope = self.scope_name
        self._proc_insts[self.root_scope_name][inst_proc_idx].append(inst)
        if getattr(inst, "gen_mode", 0) == 1 and inst_proc_idx != eng_proc_idx:
            eng_tick = self.global_clock.advance(eng_proc_idx)
            self.tc.prep_eng_ticks[inst.name] = (eng_proc_idx, eng_tick)
            self._prep_eng_names[self.root_scope_name].append(inst.name)


_tsa.TileClockTick._assign_tick = _assign_tick_queue_aware

F32 = mybir.dt.float32
I16 = mybir.dt.int16

# Problem constants (hardcoded per harness contract).
N = 100000   # nodes
D = 64       # feature dim
E = 1200000  # edges
C = 8        # cores
NS = N // C  # node shard = 12500
G = 8        # src groups per core (= src shards)
E_CHUNK = 1792        # edges per SWDGE instruction
K_CHUNKS = 12         # chunks per (core, group): >= max dst multiplicity (11)
E_PAD = E_CHUNK * K_CHUNKS
AGG_ROWS = 12544      # 98 * 128 destination rows incl. padding/trash region
TRASH = NS            # scatter index used for padded edges (row 12500)
LAYERS = 3
N_AGG = 4             # agg copies == SWDGE queues; chunk i on queue i%4


def build_graph(n=N, d=D, c=C, g=G, e_chunk=E_CHUNK, k_chunks=K_CHUNKS,
                layers=LAYERS):
    ns = n // c
    agg_rows = ((ns + 128) + 127) // 128 * 128
    n_tiles = agg_rows // 128
    e_pad = e_chunk * k_chunks
    ep16 = e_pad // 16
    ec16 = e_chunk // 16
    ecb = (e_chunk + 127) // 128  # msg blocks per chunk

    nc = bacc.Bacc("TRN2", target_bir_lowering=False, debug=False,
                   num_devices=c, num_swdge_queues=4)

    x_ext = nc.declare_dram_parameter("x", [n, d], F32, isOutput=False)
    g_ext = nc.declare_dram_parameter("gidx", [g, 128, ep16], I16, isOutput=False)
    s_ext = nc.declare_dram_parameter("sidx", [g, 128, ep16], I16, isOutput=False)
    w_ext = nc.declare_dram_parameter("w", [layers, d, d], F32, isOutput=False)
    out_ext = nc.declare_dram_parameter("out", [ns, d], F32, isOutput=True)

    aggs = [nc.dram_tensor(f"agg{q}", [agg_rows, d], F32) for q in range(N_AGG)]
    shards = [nc.dram_tensor(f"shard{l}", [ns, d], F32) for l in range(layers - 1)]
    tables = [
        nc.dram_tensor(f"table{l}", [n, d], F32, addr_space="Shared")
        for l in range(1, layers)
    ]

    with tile.TileContext(nc) as tc:
        with (
            tc.tile_pool(name="const", bufs=1) as cpool,
            tc.tile_pool(name="idx", bufs=1) as ipool,
            tc.tile_pool(name="msg", bufs=8) as mpool,
            tc.tile_pool(name="gemm", bufs=4) as gpool,
            tc.tile_pool(name="psum", bufs=4, space="PSUM") as ppool,
        ):
            ident = cpool.tile([128, 128], F32)
            make_identity(nc, ident[:])

            w_sb = cpool.tile([d, layers * d], F32)
            for l in range(layers):
                nc.sync.dma_start(out=w_sb[:, l * d:(l + 1) * d], in_=w_ext[l])

            zero_sb = cpool.tile([128, agg_rows * d // 128], F32)
            nc.vector.memset(zero_sb[:], 0.0)

            gidx_sb, sidx_sb = [], []
            for gi in range(g):
                gt = ipool.tile([128, ep16], I16, name=f"gidx{gi}")
                nc.sync.dma_start(out=gt[:], in_=g_ext[gi])
                st = ipool.tile([128, ep16], I16, name=f"sidx{gi}")
                nc.sync.dma_start(out=st[:], in_=s_ext[gi])
                gidx_sb.append(gt)
                sidx_sb.append(st)

            for l in range(layers):
                table = x_ext if l == 0 else tables[l - 1]

                for q in range(N_AGG):
                    nc.sync.dma_start(
                        out=aggs[q][:].rearrange("(p b) e -> p (b e)", p=128),
                        in_=zero_sb[:],
                    )

                chunks = [(gi, ki) for gi in range(g) for ki in range(k_chunks)]
                rounds = [chunks[r0:r0 + N_AGG]
                          for r0 in range(0, len(chunks), N_AGG)]

                def emit_gathers(rc):
                    msgs = []
                    for q, (gi, ki) in enumerate(rc):
                        msg = mpool.tile([128, ecb * d], F32,
                                         name="msg", tag=f"msg{q}")
                        nc.gpsimd.dma_gather(
                            out_ap=msg[:].rearrange("p (b e) -> p b e", e=d),
                            in_ap=table[gi * ns:(gi + 1) * ns, :],
                            idxs_ap=gidx_sb[gi][:, ki * ec16:(ki + 1) * ec16],
                            num_idxs=e_chunk,
                            num_idxs_reg=e_chunk,
                            elem_size=d,
                            queue_num=q,
                            single_packet=False,
                        )
                        msgs.append(msg)
                    return msgs

                def emit_scatters(rc, msgs):
                    for q, (gi, ki) in enumerate(rc):
                        nc.gpsimd.dma_scatter_add(
                            out_ap=aggs[q][:, :],
                            in_ap=msgs[q][:].rearrange("p (b e) -> p b e", e=d),
                            idxs_ap=sidx_sb[gi][:, ki * ec16:(ki + 1) * ec16],
                            num_idxs=e_chunk,
                            num_idxs_reg=e_chunk,
                            elem_size=d,
                            queue_num=q,
                            single_packet=False,
                        )

                # software pipeline: gathers run one round ahead of scatters
                prev = None
                for rc in rounds:
                    msgs = emit_gathers(rc)
                    if prev is not None:
                        emit_scatters(*prev)
                    prev = (rc, msgs)
                emit_scatters(*prev)

                dest = out_ext if l == layers - 1 else shards[l]
                for t in range(n_tiles):
                    a = gpool.tile([128, d], F32, name="a", tag="a")
                    nc.sync.dma_start(out=a[:], in_=aggs[0][t * 128:(t + 1) * 128, :])
                    for qq in range(1, N_AGG):
                        a1 = gpool.tile([128, d], F32, name="a1", tag="a1")
                        nc.sync.dma_start(out=a1[:], in_=aggs[qq][t * 128:(t + 1) * 128, :])
                        nc.any.tensor_add(out=a[:], in0=a[:], in1=a1[:])
                    at_ps = ppool.tile([d, 128], F32, name="atp", tag="atp")
                    nc.tensor.transpose(out=at_ps[:], in_=a[:], identity=ident[:])
                    at = gpool.tile([d, 128], F32, name="ats", tag="ats")
                    nc.vector.tensor_copy(out=at[:], in_=at_ps[:])
                    o_ps = ppool.tile([128, d], F32, name="ops", tag="ops")
                    nc.tensor.matmul(
                        out=o_ps[:],
                        lhsT=at[:],
                        rhs=w_sb[:, l * d:(l + 1) * d],
                        start=True,
                        stop=True,
                    )
                    o = gpool.tile([128, d], F32, name="o", tag="o")
                    nc.vector.tensor_copy(out=o[:], in_=o_ps[:])
                    rows = min(128, ns - t * 128)
                    if rows > 0:
                        nc.sync.dma_start(
                            out=dest[t * 128:t * 128 + rows, :], in_=o[:rows, :]
                        )

                if l < layers - 1:
                    nc.gpsimd.collective_compute(
                        "AllGather",
                        mybir.AluOpType.bypass,
                        replica_groups=[list(range(c))],
                        ins=[shards[l][:]],
                        outs=[tables[l][:]],
                    )

    return nc


def _wrap16(arr_1d):
    """Wrap a [e_pad] index vector to the [16, e_pad//16] layout used by the
    gather/scatter DMA instructions (index j at partition j%16, col j//16),
    replicated across the 8 Q7-core partition groups -> [128, e_pad//16].
    Chunk c of size 2016 occupies columns [c*126, (c+1)*126)."""
    w = arr_1d.reshape(-1, 16).T
    return np.tile(w, (8, 1))


def prep_inputs(x, edge_index, W0, W1, W2, n=N, d=D, c=C, g=G,
                e_chunk=E_CHUNK, k_chunks=K_CHUNKS):
    ns = n // c
    e_pad = e_chunk * k_chunks
    src = np.asarray(edge_index[0], dtype=np.int64)
    dst = np.asarray(edge_index[1], dtype=np.int64)
    core = dst // ns
    grp = src // ns
    cg = core * g + grp
    # Sort by (group, dst); within each dst run, occurrence k goes to chunk
    # (dst + k) % k_chunks so every chunk sees each dst at most once (the
    # SDMA CCE read-modify-write races on duplicate rows within one
    # instruction; chunks are serialized against each other).
    order0 = np.argsort(cg * np.int64(n) + dst, kind="stable")
    d0 = dst[order0]
    cg0 = cg[order0]
    run_key = cg0 * np.int64(n) + d0
    new_run = np.empty(len(d0), bool)
    new_run[0] = True
    np.not_equal(run_key[1:], run_key[:-1], out=new_run[1:])
    run_id = np.cumsum(new_run) - 1
    run_start = np.flatnonzero(new_run)
    rank = np.arange(len(d0)) - run_start[run_id]
    assert rank.max() < k_chunks, f"dst multiplicity {rank.max()+1} > k_chunks"
    chunk = (d0 + rank) % k_chunks
    order1 = np.argsort(cg0 * np.int64(k_chunks) + chunk, kind="stable")
    s_src = src[order0][order1]
    s_dst = d0[order1]
    s_cgk = (cg0 * k_chunks + chunk)[order1]
    counts = np.bincount(s_cgk, minlength=c * g * k_chunks)
    assert counts.max() <= e_chunk, f"chunk count {counts.max()} > {e_chunk}"
    offsets = np.zeros(c * g * k_chunks + 1, np.int64)
    np.cumsum(counts, out=offsets[1:])

    x = np.ascontiguousarray(np.asarray(x, dtype=np.float32))
    w = np.ascontiguousarray(np.stack([W0, W1, W2]).astype(np.float32))

    in_maps = []
    ep16 = e_pad // 16
    for ci in range(c):
        gidx = np.zeros((g, 128, ep16), np.int16)
        sidx = np.zeros((g, 128, ep16), np.int16)
        for gi in range(g):
            ga = np.zeros(e_pad, np.int16)
            sa = np.full(e_pad, ns, np.int16)  # trash row
            for ki in range(k_chunks):
                j = (ci * g + gi) * k_chunks + ki
                lo, hi = offsets[j], offsets[j + 1]
                nseg = hi - lo
                base = ki * e_chunk
                ga[base:base + nseg] = s_src[lo:hi] - gi * ns
                sa[base:base + nseg] = s_dst[lo:hi] - ci * ns
            gidx[gi] = _wrap16(ga)
            sidx[gi] = _wrap16(sa)
        in_maps.append({"x": x, "gidx": gidx, "sidx": sidx, "w": w})
    return in_maps


def kernel(x, edge_index, W0, W1, W2, _trace=False, _trace_kwargs=None):
    nc = build_graph()
    nc.compile()
    in_maps = prep_inputs(x, edge_index, W0, W1, W2)
    res = run_bass_kernel_spmd(
        nc, in_maps, core_ids=list(range(C)), trace=_trace,
        **(_trace_kwargs or {}),
    )
    out = np.concatenate([res.results[ci]["out"] for ci in range(C)], axis=0)
    if _trace:
        return out, res
    return out


# revision 14
# speedup vs baseline: 1.5575x; 1.3834x over previous
"""3-layer GCN (GCNConv normalize=False, bias=False, aggr='add') on 8 TRN2 NeuronCores.

Math: each layer computes segment_sum((x @ W)[src], dst) which equals
segment_sum(x[src], dst) @ W (linear ops commute), so we aggregate the raw
features first (gather + scatter-add over edges) and then apply the tiny 64x64
weight to the 12500-row destination shard only.

Distribution (8 cores):
  - nodes sharded by id: core c owns dst rows [c*12500, (c+1)*12500)
  - edges assigned to the core owning their dst; within a core, edges are
    grouped by src shard (8 groups) so gather indices fit in int16 against a
    12500-row subtable, and sorted by dst inside each group
  - per layer: dma_gather rows of the full node-feature table (x for layer 0,
    AllGather output for layers 1/2) -> SBUF, dma_scatter_add into an agg
    table in DRAM, small GEMM agg @ W_l, AllGather the shard for the next
    layer. Final-layer shards are concatenated on the host.

SWDGE details: one gather/scatter instruction is limited to ~<=2032 indices
(descriptor ring is 128 slots/engine; descs = num_idxs/16+1), so edges are
chunked at 2016/instruction. Gathers run on SWDGE queues 0/1, scatters on
queues 2/3 into two separate agg copies (queue q -> Q7 core pair q; separate
agg copies keep concurrent scatter-add streams race-free). The GEMM sums the
two copies.
"""

import numpy as np

from concourse import bacc, bass, mybir, tile
from concourse import tile_sem_assignment as _tsa
from concourse.bass_utils import run_bass_kernel_spmd
from concourse.masks import make_identity


def _assign_tick_queue_aware(self, inst):
    """Queue-aware replacement for TileClockTick._assign_tick: SWDGE (Pool)
    DMA instructions get DMASW semaphore lanes partitioned by their SWDGE
    queue_num (lanes {2q, 2q+1} for queue q) instead of global round-robin.
    The runtime locks each DMASW semaphore to the first SWDGE queue that
    uses it, so cross-queue round-robin deadlocks/corrupts. Everything else
    is identical to the original (concourse/tile_sem_assignment.py)."""
    engine = inst.engine
    eng_proc_idx = (
        _tsa.ENGINE_SEQUENCER_TO_IDX if inst.is_sequencer_only()
        else _tsa.ENGINE_TO_IDX
    )[engine]
    if isinstance(inst, _tsa.DMAInst) and not isinstance(
        inst, _tsa.bass_isa.UserSyncedRemoteDMADescs
    ):
        if engine == _tsa.mybir.EngineType.Pool:
            if self.swdge_sem_count == _tsa.NUM_SWDGE_GLOBAL_SEMS:
                qn = int(getattr(inst, "queue_num", 0) or 0) % 4
                lanes_per_q = _tsa.NUM_SWDGE_GLOBAL_SEMS // 4
                tog = getattr(self, "_q_lane_toggle", None)
                if tog is None:
                    tog = self._q_lane_toggle = {}
                t = tog.get(qn, 0)
                tog[qn] = (t + 1) % lanes_per_q
                lane = qn * lanes_per_q + t
            else:
                lane = self.next_sw_dma_idx
                self.next_sw_dma_idx = (
                    self.next_sw_dma_idx + 1
                ) % self.swdge_sem_count
            inst_proc_idx = _tsa.PROC_NAME_TO_IDX[f"DMASW{lane}"]
        else:
            inst_proc_idx = _tsa.PROC_NAME_TO_IDX[f"DMAHW{self.next_hw_dma_idx}"]
            self.next_hw_dma_idx = (self.next_hw_dma_idx + 1) % _tsa.NUM_HWDGE_SEMS
    elif isinstance(inst, _tsa.mybir.InstCollectiveCompute):
        inst_proc_idx = _tsa.PROC_NAME_TO_IDX["Collectives"]
    else:
        inst_proc_idx = eng_proc_idx

    if not inst.is_executable():
        if not isinstance(inst, _tsa.BassTileCriticalSection):
            return
    if isinstance(inst, _tsa.bass_isa.InstPseudoReloadLibraryIndex):
        return

    if inst.descendants or isinstance(inst, _tsa._DMA_OR_COLLECTIVE_TYPES):
        inst.bass_scheduled_tick = self.global_clock.advance(inst_proc_idx)
        inst.bass_scheduled_proc = inst_proc_idx
        inst.bass_scheduled_scope = self.scope_name
        self._proc_insts[self.root_scope_name][inst_proc_idx].append(inst)
        if getattr(inst, "gen_mode", 0) == 1 and inst_proc_idx != eng_proc_idx:
            eng_tick = self.global_clock.advance(eng_proc_idx)
            self.tc.prep_eng_ticks[inst.name] = (eng_proc_idx, eng_tick)
            self._prep_eng_names[self.root_scope_name].append(inst.name)


_tsa.TileClockTick._assign_tick = _assign_tick_queue_aware

F32 = mybir.dt.float32
I16 = mybir.dt.int16

# Problem constants (hardcoded per harness contract).
N = 100000   # nodes
D = 64       # feature dim
E = 1200000  # edges
C = 8        # cores
NS = N // C  # node shard = 12500
G = 8        # src groups per core (= src shards)
E_CHUNK = 1792        # edges per SWDGE instruction
K_CHUNKS = 12         # chunks per (core, group): >= max dst multiplicity (11)
E_PAD = E_CHUNK * K_CHUNKS
AGG_ROWS = 12544      # 98 * 128 destination rows incl. padding/trash region
TRASH = NS            # scatter index used for padded edges (row 12500)
LAYERS = 3
N_AGG = 4             # agg copies == SWDGE queues; chunk i on queue i%4


def build_graph(n=N, d=D, c=C, g=G, e_chunk=E_CHUNK, k_chunks=K_CHUNKS,
                layers=LAYERS, chunk_sizes=None):
    ns = n // c
    agg_rows = ((ns + 128) + 127) // 128 * 128
    n_tiles = agg_rows // 128
    if chunk_sizes is None:
        chunk_sizes = [e_chunk] * k_chunks
    assert len(chunk_sizes) == k_chunks
    chunk_off = [0]
    for s in chunk_sizes:
        assert s % 16 == 0
        chunk_off.append(chunk_off[-1] + s)
    e_pad = chunk_off[-1]
    ep16 = e_pad // 16
    ecb_max = (max(chunk_sizes) + 127) // 128

    nc = bacc.Bacc("TRN2", target_bir_lowering=False, debug=False,
                   num_devices=c, num_swdge_queues=4)

    x_ext = nc.declare_dram_parameter("x", [n, d], F32, isOutput=False)
    g_ext = nc.declare_dram_parameter("gidx", [g, 128, ep16], I16, isOutput=False)
    s_ext = nc.declare_dram_parameter("sidx", [g, 128, ep16], I16, isOutput=False)
    w_ext = nc.declare_dram_parameter("w", [layers, d, d], F32, isOutput=False)
    out_ext = nc.declare_dram_parameter("out", [ns, d], F32, isOutput=True)

    aggs = [nc.dram_tensor(f"agg{q}", [agg_rows, d], F32) for q in range(N_AGG)]
    shards = [nc.dram_tensor(f"shard{l}", [ns, d], F32) for l in range(layers - 1)]
    tables = [
        nc.dram_tensor(f"table{l}", [n, d], F32, addr_space="Shared")
        for l in range(1, layers)
    ]

    with tile.TileContext(nc) as tc:
        with (
            tc.tile_pool(name="const", bufs=1) as cpool,
            tc.tile_pool(name="idx", bufs=1) as ipool,
            tc.tile_pool(name="msg", bufs=8) as mpool,
            tc.tile_pool(name="gemm", bufs=4) as gpool,
            tc.tile_pool(name="psum", bufs=4, space="PSUM") as ppool,
        ):
            ident = cpool.tile([128, 128], F32)
            make_identity(nc, ident[:])

            w_sb = cpool.tile([d, layers * d], F32)
            for l in range(layers):
                nc.sync.dma_start(out=w_sb[:, l * d:(l + 1) * d], in_=w_ext[l])

            zero_sb = cpool.tile([128, agg_rows * d // 128], F32)
            nc.vector.memset(zero_sb[:], 0.0)

            gidx_sb, sidx_sb = [], []
            for gi in range(g):
                gt = ipool.tile([128, ep16], I16, name=f"gidx{gi}")
                nc.sync.dma_start(out=gt[:], in_=g_ext[gi])
                st = ipool.tile([128, ep16], I16, name=f"sidx{gi}")
                nc.sync.dma_start(out=st[:], in_=s_ext[gi])
                gidx_sb.append(gt)
                sidx_sb.append(st)

            for l in range(layers):
                table = x_ext if l == 0 else tables[l - 1]

                for q in range(N_AGG):
                    nc.sync.dma_start(
                        out=aggs[q][:].rearrange("(p b) e -> p (b e)", p=128),
                        in_=zero_sb[:],
                    )

                chunks = [(gi, ki) for gi in range(g) for ki in range(k_chunks)]
                rounds = [chunks[r0:r0 + N_AGG]
                          for r0 in range(0, len(chunks), N_AGG)]

                def emit_gathers(rc):
                    msgs = []
                    for q, (gi, ki) in enumerate(rc):
                        ec = chunk_sizes[ki]
                        ecb = (ec + 127) // 128
                        msg = mpool.tile([128, ecb_max * d], F32,
                                         name="msg", tag=f"msg{q}")
                        nc.gpsimd.dma_gather(
                            out_ap=msg[:, :ecb * d].rearrange(
                                "p (b e) -> p b e", e=d),
                            in_ap=table[gi * ns:(gi + 1) * ns, :],
                            idxs_ap=gidx_sb[gi][:, chunk_off[ki] // 16:
                                               chunk_off[ki + 1] // 16],
                            num_idxs=ec,
                            num_idxs_reg=ec,
                            elem_size=d,
                            queue_num=q,
                            single_packet=False,
                        )
                        msgs.append(msg)
                    return msgs

                def emit_scatters(rc, msgs):
                    for q, (gi, ki) in enumerate(rc):
                        ec = chunk_sizes[ki]
                        ecb = (ec + 127) // 128
                        nc.gpsimd.dma_scatter_add(
                            out_ap=aggs[q][:, :],
                            in_ap=msgs[q][:, :ecb * d].rearrange(
                                "p (b e) -> p b e", e=d),
                            idxs_ap=sidx_sb[gi][:, chunk_off[ki] // 16:
                                               chunk_off[ki + 1] // 16],
                            num_idxs=ec,
                            num_idxs_reg=ec,
                            elem_size=d,
                            queue_num=q,
                            single_packet=False,
                        )

                # software pipeline: gathers run one round ahead of scatters
                prev = None
                for rc in rounds:
                    msgs = emit_gathers(rc)
                    if prev is not None:
                        emit_scatters(*prev)
                    prev = (rc, msgs)
                emit_scatters(*prev)

                dest = out_ext if l == layers - 1 else shards[l]
                for t in range(n_tiles):
                    a = gpool.tile([128, d], F32, name="a", tag="a")
                    nc.sync.dma_start(out=a[:], in_=aggs[0][t * 128:(t + 1) * 128, :])
                    for qq in range(1, N_AGG):
                        a1 = gpool.tile([128, d], F32, name="a1", tag="a1")
                        nc.sync.dma_start(out=a1[:], in_=aggs[qq][t * 128:(t + 1) * 128, :])
                        nc.any.tensor_add(out=a[:], in0=a[:], in1=a1[:])
                    at_ps = ppool.tile([d, 128], F32, name="atp", tag="atp")
                    nc.tensor.transpose(out=at_ps[:], in_=a[:], identity=ident[:])
                    at = gpool.tile([d, 128], F32, name="ats", tag="ats")
                    nc.vector.tensor_copy(out=at[:], in_=at_ps[:])
                    o_ps = ppool.tile([128, d], F32, name="ops", tag="ops")
                    nc.tensor.matmul(
                        out=o_ps[:],
                        lhsT=at[:],
                        rhs=w_sb[:, l * d:(l + 1) * d],
                        start=True,
                        stop=True,
                    )
                    o = gpool.tile([128, d], F32, name="o", tag="o")
                    nc.vector.tensor_copy(out=o[:], in_=o_ps[:])
                    rows = min(128, ns - t * 128)
                    if rows > 0:
                        nc.sync.dma_start(
                            out=dest[t * 128:t * 128 + rows, :], in_=o[:rows, :]
                        )

                if l < layers - 1:
                    nc.gpsimd.collective_compute(
                        "AllGather",
                        mybir.AluOpType.bypass,
                        replica_groups=[list(range(c))],
                        ins=[shards[l][:]],
                        outs=[tables[l][:]],
                    )

    return nc


def _wrap16(arr_1d):
    """Wrap a [e_pad] index vector to the [16, e_pad//16] layout used by the
    gather/scatter DMA instructions (index j at partition j%16, col j//16),
    replicated across the 8 Q7-core partition groups -> [128, e_pad//16].
    Chunk c of size 2016 occupies columns [c*126, (c+1)*126)."""
    w = arr_1d.reshape(-1, 16).T
    return np.tile(w, (8, 1))


def prep_inputs(x, edge_index, W0, W1, W2, n=N, d=D, c=C, g=G,
                e_chunk=E_CHUNK, k_chunks=K_CHUNKS):
    ns = n // c
    src = np.asarray(edge_index[0], dtype=np.int64)
    dst = np.asarray(edge_index[1], dtype=np.int64)
    core = dst // ns
    grp = src // ns
    cg = core * g + grp
    # Sort by (group, dst); within each dst run, occurrence k goes to chunk
    # (dst + k) % k_chunks so every chunk sees each dst at most once (the
    # SDMA CCE read-modify-write races on duplicate rows within one
    # instruction; chunks are serialized against each other).
    order0 = np.argsort(cg * np.int64(n) + dst, kind="stable")
    d0 = dst[order0]
    cg0 = cg[order0]
    run_key = cg0 * np.int64(n) + d0
    new_run = np.empty(len(d0), bool)
    new_run[0] = True
    np.not_equal(run_key[1:], run_key[:-1], out=new_run[1:])
    run_id = np.cumsum(new_run) - 1
    run_start = np.flatnonzero(new_run)
    rank = np.arange(len(d0)) - run_start[run_id]
    assert rank.max() < k_chunks, f"dst multiplicity {rank.max()+1} > k_chunks"
    chunk = (d0 + rank) % k_chunks
    order1 = np.argsort(cg0 * np.int64(k_chunks) + chunk, kind="stable")
    s_src = src[order0][order1]
    s_dst = d0[order1]
    s_cgk = (cg0 * k_chunks + chunk)[order1]
    counts = np.bincount(s_cgk, minlength=c * g * k_chunks)
    # static per-ki chunk size = max load over all (core, group), 16-aligned
    loads = counts.reshape(c * g, k_chunks)
    chunk_sizes = [int(-(-int(loads[:, ki].max()) // 16) * 16) or 16
                   for ki in range(k_chunks)]
    chunk_off = [0]
    for s in chunk_sizes:
        chunk_off.append(chunk_off[-1] + s)
    e_pad = chunk_off[-1]
    offsets = np.zeros(c * g * k_chunks + 1, np.int64)
    np.cumsum(counts, out=offsets[1:])

    x = np.ascontiguousarray(np.asarray(x, dtype=np.float32))
    w = np.ascontiguousarray(np.stack([W0, W1, W2]).astype(np.float32))

    in_maps = []
    ep16 = e_pad // 16
    for ci in range(c):
        gidx = np.zeros((g, 128, ep16), np.int16)
        sidx = np.zeros((g, 128, ep16), np.int16)
        for gi in range(g):
            ga = np.zeros(e_pad, np.int16)
            sa = np.full(e_pad, ns, np.int16)  # trash row
            for ki in range(k_chunks):
                j = (ci * g + gi) * k_chunks + ki
                lo, hi = offsets[j], offsets[j + 1]
                nseg = hi - lo
                base = chunk_off[ki]
                ga[base:base + nseg] = s_src[lo:hi] - gi * ns
                sa[base:base + nseg] = s_dst[lo:hi] - ci * ns
            gidx[gi] = _wrap16(ga)
            sidx[gi] = _wrap16(sa)
        in_maps.append({"x": x, "gidx": gidx, "sidx": sidx, "w": w})
    return in_maps, chunk_sizes


def kernel(x, edge_index, W0, W1, W2, _trace=False, _trace_kwargs=None):
    in_maps, chunk_sizes = prep_inputs(x, edge_index, W0, W1, W2)
    nc = build_graph(chunk_sizes=chunk_sizes)
    nc.compile()
    res = run_bass_kernel_spmd(
        nc, in_maps, core_ids=list(range(C)), trace=_trace,
        **(_trace_kwargs or {}),
    )
    out = np.concatenate([res.results[ci]["out"] for ci in range(C)], axis=0)
    if _trace:
        return out, res
    return out


# revision 17
# speedup vs baseline: 1.7283x; 1.1097x over previous
"""3-layer GCN (GCNConv normalize=False, bias=False, aggr='add') on 8 TRN2 NeuronCores.

Math: each layer computes segment_sum((x @ W)[src], dst) which equals
segment_sum(x[src], dst) @ W (linear ops commute), so we aggregate the raw
features first (gather + scatter-add over edges) and then apply the tiny 64x64
weight to the 12500-row destination shard only.

Distribution (8 cores):
  - nodes sharded by id: core c owns dst rows [c*12500, (c+1)*12500)
  - edges assigned to the core owning their dst; within a core, edges are
    grouped by src shard (8 groups) so gather indices fit in int16 against a
    12500-row subtable, and sorted by dst inside each group
  - per layer: dma_gather rows of the full node-feature table (x for layer 0,
    AllGather output for layers 1/2) -> SBUF, dma_scatter_add into an agg
    table in DRAM, small GEMM agg @ W_l, AllGather the shard for the next
    layer. Final-layer shards are concatenated on the host.

SWDGE details: one gather/scatter instruction is limited to ~<=2032 indices
(descriptor ring is 128 slots/engine; descs = num_idxs/16+1), so edges are
chunked at 2016/instruction. Gathers run on SWDGE queues 0/1, scatters on
queues 2/3 into two separate agg copies (queue q -> Q7 core pair q; separate
agg copies keep concurrent scatter-add streams race-free). The GEMM sums the
two copies.
"""

import numpy as np

from concourse import bacc, bass, mybir, tile
from concourse import tile_sem_assignment as _tsa
from concourse.bass_utils import run_bass_kernel_spmd
from concourse.masks import make_identity


def _assign_tick_queue_aware(self, inst):
    """Queue-aware replacement for TileClockTick._assign_tick: SWDGE (Pool)
    DMA instructions get DMASW semaphore lanes partitioned by their SWDGE
    queue_num (lanes {2q, 2q+1} for queue q) instead of global round-robin.
    The runtime locks each DMASW semaphore to the first SWDGE queue that
    uses it, so cross-queue round-robin deadlocks/corrupts. Everything else
    is identical to the original (concourse/tile_sem_assignment.py)."""
    engine = inst.engine
    eng_proc_idx = (
        _tsa.ENGINE_SEQUENCER_TO_IDX if inst.is_sequencer_only()
        else _tsa.ENGINE_TO_IDX
    )[engine]
    if isinstance(inst, _tsa.DMAInst) and not isinstance(
        inst, _tsa.bass_isa.UserSyncedRemoteDMADescs
    ):
        if engine == _tsa.mybir.EngineType.Pool:
            if self.swdge_sem_count == _tsa.NUM_SWDGE_GLOBAL_SEMS:
                qn = int(getattr(inst, "queue_num", 0) or 0) % 4
                lanes_per_q = _tsa.NUM_SWDGE_GLOBAL_SEMS // 4
                tog = getattr(self, "_q_lane_toggle", None)
                if tog is None:
                    tog = self._q_lane_toggle = {}
                t = tog.get(qn, 0)
                tog[qn] = (t + 1) % lanes_per_q
                lane = qn * lanes_per_q + t
            else:
                lane = self.next_sw_dma_idx
                self.next_sw_dma_idx = (
                    self.next_sw_dma_idx + 1
                ) % self.swdge_sem_count
            inst_proc_idx = _tsa.PROC_NAME_TO_IDX[f"DMASW{lane}"]
        else:
            inst_proc_idx = _tsa.PROC_NAME_TO_IDX[f"DMAHW{self.next_hw_dma_idx}"]
            self.next_hw_dma_idx = (self.next_hw_dma_idx + 1) % _tsa.NUM_HWDGE_SEMS
    elif isinstance(inst, _tsa.mybir.InstCollectiveCompute):
        inst_proc_idx = _tsa.PROC_NAME_TO_IDX["Collectives"]
    else:
        inst_proc_idx = eng_proc_idx

    if not inst.is_executable():
        if not isinstance(inst, _tsa.BassTileCriticalSection):
            return
    if isinstance(inst, _tsa.bass_isa.InstPseudoReloadLibraryIndex):
        return

    if inst.descendants or isinstance(inst, _tsa._DMA_OR_COLLECTIVE_TYPES):
        inst.bass_scheduled_tick = self.global_clock.advance(inst_proc_idx)
        inst.bass_scheduled_proc = inst_proc_idx
        inst.bass_scheduled_scope = self.scope_name
        self._proc_insts[self.root_scope_name][inst_proc_idx].append(inst)
        if getattr(inst, "gen_mode", 0) == 1 and inst_proc_idx != eng_proc_idx:
            eng_tick = self.global_clock.advance(eng_proc_idx)
            self.tc.prep_eng_ticks[inst.name] = (eng_proc_idx, eng_tick)
            self._prep_eng_names[self.root_scope_name].append(inst.name)


_tsa.TileClockTick._assign_tick = _assign_tick_queue_aware

F32 = mybir.dt.float32
I16 = mybir.dt.int16

# Problem constants (hardcoded per harness contract).
N = 100000   # nodes
D = 64       # feature dim
E = 1200000  # edges
C = 8        # cores
NS = N // C  # node shard = 12500
G = 8        # src groups per core (= src shards)
E_CHUNK = 1792        # edges per SWDGE instruction
K_CHUNKS = 12         # chunks per (core, group): >= max dst multiplicity (11)
E_PAD = E_CHUNK * K_CHUNKS
AGG_ROWS = 12544      # 98 * 128 destination rows incl. padding/trash region
TRASH = NS            # scatter index used for padded edges (row 12500)
LAYERS = 3
N_AGG = 4             # agg copies == SWDGE queues; chunk i on queue i%4


def build_graph(n=N, d=D, c=C, g=G, e_chunk=E_CHUNK, k_chunks=K_CHUNKS,
                layers=LAYERS, chunk_sizes=None):
    ns = n // c
    agg_rows = ((ns + 128) + 127) // 128 * 128
    n_tiles = agg_rows // 128
    if chunk_sizes is None:
        chunk_sizes = [e_chunk] * k_chunks
    assert len(chunk_sizes) == k_chunks
    chunk_off = [0]
    for s in chunk_sizes:
        assert s % 16 == 0
        chunk_off.append(chunk_off[-1] + s)
    e_pad = chunk_off[-1]
    ep16 = e_pad // 16
    ecb_max = (max(chunk_sizes) + 127) // 128

    nc = bacc.Bacc("TRN2", target_bir_lowering=False, debug=False,
                   num_devices=c, num_swdge_queues=4)

    x_ext = nc.declare_dram_parameter("x", [n, d], F32, isOutput=False)
    g_ext = nc.declare_dram_parameter("gidx", [g, 128, ep16], I16, isOutput=False)
    s_ext = nc.declare_dram_parameter("sidx", [g, 128, ep16], I16, isOutput=False)
    w_ext = nc.declare_dram_parameter("w", [layers, d, d], F32, isOutput=False)
    out_ext = nc.declare_dram_parameter("out", [ns, d], F32, isOutput=True)

    aggs = [nc.dram_tensor(f"agg{q}", [agg_rows, d], F32) for q in range(N_AGG)]
    shards = [nc.dram_tensor(f"shard{l}", [ns, d], F32) for l in range(layers - 1)]
    tables = [
        nc.dram_tensor(f"table{l}", [n, d], F32, addr_space="Shared")
        for l in range(1, layers)
    ]

    with tile.TileContext(nc) as tc:
        with (
            tc.tile_pool(name="const", bufs=1) as cpool,
            tc.tile_pool(name="idx", bufs=1) as ipool,
            tc.tile_pool(name="msg", bufs=8) as mpool,
            tc.tile_pool(name="gemm", bufs=16) as gpool,
            tc.tile_pool(name="psum", bufs=4, space="PSUM") as ppool,
        ):
            ident = cpool.tile([128, 128], F32)
            make_identity(nc, ident[:])

            w_sb = cpool.tile([d, layers * d], F32)
            for l in range(layers):
                nc.sync.dma_start(out=w_sb[:, l * d:(l + 1) * d], in_=w_ext[l])

            zero_sb = cpool.tile([128, agg_rows * d // 128], F32)
            nc.vector.memset(zero_sb[:], 0.0)

            gidx_sb, sidx_sb = [], []
            for gi in range(g):
                gt = ipool.tile([128, ep16], I16, name=f"gidx{gi}")
                nc.sync.dma_start(out=gt[:], in_=g_ext[gi])
                st = ipool.tile([128, ep16], I16, name=f"sidx{gi}")
                nc.sync.dma_start(out=st[:], in_=s_ext[gi])
                gidx_sb.append(gt)
                sidx_sb.append(st)

            for l in range(layers):
                table = x_ext if l == 0 else tables[l - 1]

                for q in range(N_AGG):
                    nc.sync.dma_start(
                        out=aggs[q][:].rearrange("(p b) e -> p (b e)", p=128),
                        in_=zero_sb[:],
                    )

                chunks = [(gi, ki) for gi in range(g) for ki in range(k_chunks)]
                rounds = [chunks[r0:r0 + N_AGG]
                          for r0 in range(0, len(chunks), N_AGG)]

                def emit_gathers(rc):
                    msgs = []
                    for q, (gi, ki) in enumerate(rc):
                        ec = chunk_sizes[ki]
                        ecb = (ec + 127) // 128
                        msg = mpool.tile([128, ecb_max * d], F32,
                                         name="msg", tag=f"msg{q}")
                        nc.gpsimd.dma_gather(
                            out_ap=msg[:, :ecb * d].rearrange(
                                "p (b e) -> p b e", e=d),
                            in_ap=table[gi * ns:(gi + 1) * ns, :],
                            idxs_ap=gidx_sb[gi][:, chunk_off[ki] // 16:
                                               chunk_off[ki + 1] // 16],
                            num_idxs=ec,
                            num_idxs_reg=ec,
                            elem_size=d,
                            queue_num=q,
                            single_packet=False,
                        )
                        msgs.append(msg)
                    return msgs

                def emit_scatters(rc, msgs):
                    for q, (gi, ki) in enumerate(rc):
                        ec = chunk_sizes[ki]
                        ecb = (ec + 127) // 128
                        nc.gpsimd.dma_scatter_add(
                            out_ap=aggs[q][:, :],
                            in_ap=msgs[q][:, :ecb * d].rearrange(
                                "p (b e) -> p b e", e=d),
                            idxs_ap=sidx_sb[gi][:, chunk_off[ki] // 16:
                                               chunk_off[ki + 1] // 16],
                            num_idxs=ec,
                            num_idxs_reg=ec,
                            elem_size=d,
                            queue_num=q,
                            single_packet=False,
                        )

                # software pipeline: gathers run one round ahead of scatters
                prev = None
                for rc in rounds:
                    msgs = emit_gathers(rc)
                    if prev is not None:
                        emit_scatters(*prev)
                    prev = (rc, msgs)
                emit_scatters(*prev)

                dest = out_ext if l == layers - 1 else shards[l]
                for t in range(n_tiles):
                    # transpose-accumulate the N_AGG copies into one PSUM tile:
                    # at_ps = sum_q agg_q[tile]^T
                    at_ps = ppool.tile([d, 128], F32, name="atp", tag="atp")
                    for qq in range(N_AGG):
                        a = gpool.tile([128, d], F32, name="a", tag=f"a{qq}")
                        nc.sync.dma_start(
                            out=a[:], in_=aggs[qq][t * 128:(t + 1) * 128, :])
                        nc.tensor.matmul(
                            out=at_ps[:],
                            lhsT=a[:],
                            rhs=ident[:],
                            is_transpose=True,
                            start=(qq == 0),
                            stop=(qq == N_AGG - 1),
                        )
                    at = gpool.tile([d, 128], F32, name="ats", tag="ats")
                    nc.vector.tensor_copy(out=at[:], in_=at_ps[:])
                    o_ps = ppool.tile([128, d], F32, name="ops", tag="ops")
                    nc.tensor.matmul(
                        out=o_ps[:],
                        lhsT=at[:],
                        rhs=w_sb[:, l * d:(l + 1) * d],
                        start=True,
                        stop=True,
                    )
                    o = gpool.tile([128, d], F32, name="o", tag="o")
                    nc.any.tensor_copy(out=o[:], in_=o_ps[:])
                    rows = min(128, ns - t * 128)
                    if rows > 0:
                        nc.sync.dma_start(
                            out=dest[t * 128:t * 128 + rows, :], in_=o[:rows, :]
                        )

                if l < layers - 1:
                    nc.gpsimd.collective_compute(
                        "AllGather",
                        mybir.AluOpType.bypass,
                        replica_groups=[list(range(c))],
                        ins=[shards[l][:]],
                        outs=[tables[l][:]],
                    )

    return nc


def _wrap16(arr_1d):
    """Wrap a [e_pad] index vector to the [16, e_pad//16] layout used by the
    gather/scatter DMA instructions (index j at partition j%16, col j//16),
    replicated across the 8 Q7-core partition groups -> [128, e_pad//16].
    Chunk c of size 2016 occupies columns [c*126, (c+1)*126)."""
    w = arr_1d.reshape(-1, 16).T
    return np.tile(w, (8, 1))


def prep_inputs(x, edge_index, W0, W1, W2, n=N, d=D, c=C, g=G,
                e_chunk=E_CHUNK, k_chunks=K_CHUNKS):
    ns = n // c
    src = np.asarray(edge_index[0], dtype=np.int64)
    dst = np.asarray(edge_index[1], dtype=np.int64)
    core = dst // ns
    grp = src // ns
    cg = core * g + grp
    # Sort by (group, dst); within each dst run, occurrence k goes to chunk
    # (dst + k) % k_chunks so every chunk sees each dst at most once (the
    # SDMA CCE read-modify-write races on duplicate rows within one
    # instruction; chunks are serialized against each other).
    order0 = np.argsort(cg * np.int64(n) + dst, kind="stable")
    d0 = dst[order0]
    cg0 = cg[order0]
    run_key = cg0 * np.int64(n) + d0
    new_run = np.empty(len(d0), bool)
    new_run[0] = True
    np.not_equal(run_key[1:], run_key[:-1], out=new_run[1:])
    run_id = np.cumsum(new_run) - 1
    run_start = np.flatnonzero(new_run)
    rank = np.arange(len(d0)) - run_start[run_id]
    assert rank.max() < k_chunks, f"dst multiplicity {rank.max()+1} > k_chunks"
    chunk = (d0 + rank) % k_chunks
    order1 = np.argsort(cg0 * np.int64(k_chunks) + chunk, kind="stable")
    s_src = src[order0][order1]
    s_dst = d0[order1]
    s_cgk = (cg0 * k_chunks + chunk)[order1]
    counts = np.bincount(s_cgk, minlength=c * g * k_chunks)
    # static per-ki chunk size = max load over all (core, group), 16-aligned
    loads = counts.reshape(c * g, k_chunks)
    chunk_sizes = [int(-(-int(loads[:, ki].max()) // 16) * 16) or 16
                   for ki in range(k_chunks)]
    chunk_off = [0]
    for s in chunk_sizes:
        chunk_off.append(chunk_off[-1] + s)
    e_pad = chunk_off[-1]
    offsets = np.zeros(c * g * k_chunks + 1, np.int64)
    np.cumsum(counts, out=offsets[1:])

    x = np.ascontiguousarray(np.asarray(x, dtype=np.float32))
    w = np.ascontiguousarray(np.stack([W0, W1, W2]).astype(np.float32))

    in_maps = []
    ep16 = e_pad // 16
    for ci in range(c):
        gidx = np.zeros((g, 128, ep16), np.int16)
        sidx = np.zeros((g, 128, ep16), np.int16)
        for gi in range(g):
            ga = np.zeros(e_pad, np.int16)
            sa = np.full(e_pad, ns, np.int16)  # trash row
            for ki in range(k_chunks):
                j = (ci * g + gi) * k_chunks + ki
                lo, hi = offsets[j], offsets[j + 1]
                nseg = hi - lo
                base = chunk_off[ki]
                ga[base:base + nseg] = s_src[lo:hi] - gi * ns
                sa[base:base + nseg] = s_dst[lo:hi] - ci * ns
            gidx[gi] = _wrap16(ga)
            sidx[gi] = _wrap16(sa)
        in_maps.append({"x": x, "gidx": gidx, "sidx": sidx, "w": w})
    return in_maps, chunk_sizes


def kernel(x, edge_index, W0, W1, W2, _trace=False, _trace_kwargs=None):
    in_maps, chunk_sizes = prep_inputs(x, edge_index, W0, W1, W2)
    nc = build_graph(chunk_sizes=chunk_sizes)
    nc.compile()
    res = run_bass_kernel_spmd(
        nc, in_maps, core_ids=list(range(C)), trace=_trace,
        **(_trace_kwargs or {}),
    )
    out = np.concatenate([res.results[ci]["out"] for ci in range(C)], axis=0)
    if _trace:
        return out, res
    return out
